# revision 32
# baseline (speedup 1.0000x reference)
"""Trainium2 Bass kernel for GroupNorm + single-head spatial self-attention
(diffusion-style attention block), data-parallel on 8 NeuronCores.

Computation (per image):
    n  = GroupNorm(x; 32 groups) * gn_scale + gn_bias          [C, N]
    q  = wq @ n + bq ; k = wk @ n + bk ; v = wv @ n + bv
    A  = softmax(q^T k / sqrt(C), axis over keys)
    out = x + wp @ (A @ v)^T + bp
Shapes: B=32, C=512, H=W=32 (N = H*W = 1024 positions); 4 images/core.

Design highlights (v2):
  - Measured on this part the PE issues ~259 ns per 512-col matmul at any
    dtype (P0 downclock to ~2.0 GHz; cost = columns x accumulation
    passes).  So EVERYTHING runs fp8e4m3 DoubleRow, which halves the
    number of accumulation passes per 512-deep contraction: per image
    48 q/k/v MMs + 32 scores + 32 AV + 8 denominator + 16 proj
    (vs 192 in the f32r/fp8-mixed v1).
  - All layouts avoid transposes: S^T = k^T q in [keys, queries]; v is
    position-major so AV lands channel-major for the projection.
  - Softmax normalization deferred past AV and the projection:
    y = x + (wp @ AV_raw) * r + bp'.  The denominator is a 5th channel
    tile of AV (all-4.0 DoubleRow lhsT sums exp over keys, broadcast to
    128 partitions); one reciprocal_approx_fast gives r.
  - Scaling chain: n8 = GroupNorm(x) (unit scale), w*8 = 16*w,
    q8/k8/v8 = 16*(q,k,v), scores = 256*S so exp runs at
    scale=C^-0.5/256 with a -ln2 bias; attn-out stored x(1/64); the
    4.0-valued ones lhsT compensates everything through the shared
    denominator.  bk cancels in softmax; bv folds into bp' on host;
    bq is applied x16 at q evacuation.
  - GroupNorm scale/offset vectorized: the 4 group->channel broadcast
    matmuls land in one [128, 8] PSUM tile, and a/b come from 3 DVE ops
    on [128, 4] columns.
  - ~8.5 us of dummy bf16 warm-up matmuls at t=0 hold the PE HAM clock
    gate open through the initial DMA wait (otherwise the first ~25 us
    of real matmuls run at half clock).
  - Emission software-pipelined one image ahead; GroupNorm stats run on
    DVE/ACT/GPSIMD under the previous image's attention matmuls.
"""

import numpy as np

import concourse.bacc as bacc
import concourse.tile as tile
from concourse import mybir
from concourse import bass_utils

F32 = mybir.dt.float32
F8 = mybir.dt.float8e4
BF16 = mybir.dt.bfloat16
DR = mybir.MatmulPerfMode.DoubleRow
LN2 = 0.6931471805599453
AX = mybir.AxisListType.X
OP = mybir.AluOpType
AF = mybir.ActivationFunctionType

B, C, H, W = 32, 512, 32, 32
HW = H * W                      # 1024 spatial positions
HWH = HW // 2                   # 512 = max fp32 matmul free dim
NCORES = 8
BPC = B // NCORES               # images per core
G = 32                          # groups
GS = C // G                     # channels per group
EPS = 1e-5
P = 128
NCH = C // P                    # 4 channel chunks of 128
NPT = HW // P                   # 8 position tiles of 128
NPAIR = NCH // 2                # 2 fp8 DoubleRow channel pairs
SCALE = float(C) ** -0.5
WS = 16.0                       # fp8 weight scale
NWARM = 34                      # dummy warm-up matmuls (N=512 each)


def _build():
    nc = bacc.Bacc("TRN2", target_bir_lowering=False, debug=False)

    xs = nc.dram_tensor("xs", [BPC, C, HW], F32, kind="ExternalInput")
    wq8d = nc.dram_tensor("wq8d", [NPAIR, P, 2, C], F8, kind="ExternalInput")
    wk8d = nc.dram_tensor("wk8d", [NPAIR, P, 2, C], F8, kind="ExternalInput")
    wv8d = nc.dram_tensor("wv8d", [NPAIR, P, 2, C], F8, kind="ExternalInput")
    wp8d = nc.dram_tensor("wp8d", [NPAIR, P, 2, C], F8, kind="ExternalInput")
    # sbias columns: 0-3 gn_scale chunks, 4-7 gn_bias chunks
    sbiasd = nc.dram_tensor("sbiasd", [P, 2 * NCH], F32, kind="ExternalInput")
    bq16d = nc.dram_tensor("bq16d", [P, NCH], F32, kind="ExternalInput")
    bped = nc.dram_tensor("bped", [P, NCH], F32, kind="ExternalInput")
    gmask = nc.dram_tensor("gmask", [NCH, P, G], F32, kind="ExternalInput")
    gmaskT = nc.dram_tensor("gmaskT", [P, C], F32, kind="ExternalInput")
    ones8md = nc.dram_tensor("ones8md", [P, 2, P], F8, kind="ExternalInput")
    ys = nc.dram_tensor("ys", [BPC, C, HW], F32, kind="ExternalOutput")

    xs_ap, ys_ap = xs.ap(), ys.ap()

    with tile.TileContext(nc) as tc:
        with (
            tc.tile_pool(name="consts", bufs=1) as cp,
            tc.tile_pool(name="work", bufs=1) as wpool,
            tc.tile_pool(name="psum", bufs=2, space="PSUM") as pp,
        ):
            st_ = {}   # mutable per-image state keyed (name, b)

            # ---- image-0 x load first so GN starts before weights land ----
            def load_x(b):
                tiles = []
                engs = (nc.sync, nc.gpsimd, nc.scalar)
                for c in range(NCH):
                    xt = wpool.tile([P, HW], F32, tag=f"x{c}", bufs=2,
                                    name=f"x_b{b}_{c}")
                    engs[c % 3].dma_start(
                        out=xt, in_=xs_ap[b, c * P:(c + 1) * P, :])
                    tiles.append(xt)
                st_["x", b] = tiles

            load_x(0)

            # ---- warm-up source (no DMA dependency) + HAM hold-open ----
            wsrc = cp.tile([P, HWH], F32, tag="wsrc", name="wsrc")
            nc.vector.memset(wsrc, 0.125)
            wlhs = cp.tile([P, P], BF16, tag="wlhs", name="wlhs")
            nc.vector.tensor_copy(out=wlhs, in_=wsrc[:, :P])
            wrhs = cp.tile([P, HWH], BF16, tag="wrhs", name="wrhs")
            nc.vector.tensor_copy(out=wrhs, in_=wsrc)
            warm = pp.tile([P, HWH], F32, tag="acc1", name="warm")
            for _ in range(NWARM):
                nc.tensor.matmul(warm, lhsT=wlhs, rhs=wrhs,
                                 start=True, stop=True)

            # ---- constants ----
            def const_w8(dram, tagbase):
                tiles = []
                for j in range(NPAIR):
                    t = cp.tile([P, 2, C], F8, tag=f"{tagbase}{j}",
                                name=f"{tagbase}{j}")
                    eng = nc.sync if j % 2 == 0 else nc.gpsimd
                    eng.dma_start(out=t, in_=dram.ap()[j])
                    tiles.append(t)
                return tiles

            gm_sb = []
            for c in range(NCH):
                t = cp.tile([P, G], F32, tag=f"gm{c}", name=f"gm{c}")
                nc.sync.dma_start(out=t, in_=gmask.ap()[c])
                gm_sb.append(t)
            gmT_sb = cp.tile([P, C], F32, tag="gmT", name="gmT")
            nc.sync.dma_start(out=gmT_sb, in_=gmaskT.ap())
            sbias_sb = cp.tile([P, 2 * NCH], F32, tag="sbias", name="sbias")
            nc.gpsimd.dma_start(out=sbias_sb, in_=sbiasd.ap())
            bq16_sb = cp.tile([P, NCH], F32, tag="bq16", name="bq16")
            nc.gpsimd.dma_start(out=bq16_sb, in_=bq16d.ap())
            bpe_sb = cp.tile([P, NCH], F32, tag="bpe", name="bpe")
            nc.gpsimd.dma_start(out=bpe_sb, in_=bped.ap())
            eps_sb = cp.tile([P, 1], F32, tag="eps", name="eps")
            nc.vector.memset(eps_sb, EPS)
            zero_col = cp.tile([P, 1], F32, tag="zero", name="zero")
            nc.vector.memset(zero_col, 0.0)
            lnh_col = cp.tile([P, 1], F32, tag="lnh", name="lnh")
            nc.vector.memset(lnh_col, -LN2)

            wq_sb = const_w8(wq8d, "wq")
            wk_sb = const_w8(wk8d, "wk")
            wv_sb = const_w8(wv8d, "wv")
            wp_sb = const_w8(wp8d, "wp")
            ones8m = cp.tile([P, 2, P], F8, tag="ones8m", name="ones8m")
            nc.sync.dma_start(out=ones8m, in_=ones8md.ap())

            # ---- per-image phases ----
            def gn_stats(b):
                x_sb = st_["x", b]
                stt = []
                for c in range(NCH):
                    s = wpool.tile([P, 2], F32, tag=f"st{c}", name=f"st_b{b}_{c}")
                    nc.vector.reduce_sum(out=s[:, 0:1], in_=x_sb[c], axis=AX)
                    scr = wpool.tile([P, HW], F32, tag="sqscr", bufs=2,
                                     name=f"sqscr_b{b}_{c}")
                    nc.scalar.activation(out=scr, in_=x_sb[c], func=AF.Square,
                                         bias=zero_col, accum_out=s[:, 1:2])
                    stt.append(s)

                gp = pp.tile([G, 2], F32, tag="acc1", name=f"gp_b{b}")
                for c in range(NCH):
                    nc.tensor.matmul(gp, lhsT=gm_sb[c], rhs=stt[c],
                                     start=(c == 0), stop=(c == NCH - 1))

                # gmr: col0 = group mean, col1 = group rstd (rows >= G zero)
                gmr = wpool.tile([P, 2], F32, tag="gmr", name=f"gmr_b{b}")
                nc.vector.memset(gmr, 0.0)
                nc.vector.tensor_scalar(gmr[:G, 0:1], gp[:G, 0:1],
                                        1.0 / (GS * HW), None, OP.mult)
                m2 = wpool.tile([P, 1], F32, tag="m2", name=f"m2_b{b}")
                nc.vector.tensor_mul(m2[:G], gmr[:G, 0:1], gmr[:G, 0:1])
                var = wpool.tile([P, 1], F32, tag="var", name=f"var_b{b}")
                nc.vector.scalar_tensor_tensor(
                    out=var[:G], in0=gp[:G, 1:2], scalar=1.0 / (GS * HW),
                    in1=m2[:G], op0=OP.mult, op1=OP.subtract)
                sd = wpool.tile([P, 1], F32, tag="sd", name=f"sd_b{b}")
                nc.scalar.activation(out=sd[:G], in_=var[:G],
                                     func=AF.Sqrt, bias=eps_sb[:G])
                nc.vector.reciprocal(out=gmr[:G, 1:2], in_=sd[:G])
                st_["gmr", b] = gmr

            def normalize(b):
                x_sb, gmr = st_["x", b], st_.pop(("gmr", b))
                # one [128, 8] PSUM tile: cols (2c, 2c+1) = per-channel
                # (mean, rstd) for chunk c
                bcm = pp.tile([P, 2 * NCH], F32, tag="acc1",
                              name=f"bcm_b{b}")
                for c in range(NCH):
                    nc.tensor.matmul(bcm[:, 2 * c:2 * c + 2],
                                     lhsT=gmT_sb[:, c * P:(c + 1) * P],
                                     rhs=gmr, start=True, stop=True)
                a_all = wpool.tile([P, NCH], F32, tag="a_all",
                                   name=f"a_b{b}")
                nc.vector.tensor_mul(a_all, bcm[:, 1:2 * NCH:2],
                                     sbias_sb[:, 0:NCH])
                gt = wpool.tile([P, NCH], F32, tag="gt", name=f"gt_b{b}")
                nc.vector.tensor_mul(gt, bcm[:, 0:2 * NCH:2], a_all)
                bb = wpool.tile([P, NCH], F32, tag="bb", name=f"bb_b{b}")
                nc.vector.tensor_sub(bb, sbias_sb[:, NCH:2 * NCH], gt)
                n8 = [wpool.tile([P, 2, HW], F8, tag=f"n8{j}", bufs=2,
                                 name=f"n8_b{b}_{j}") for j in range(NPAIR)]
                for c in range(NCH):
                    neng = nc.vector if b == 0 else nc.gpsimd
                    neng.tensor_scalar(n8[c // 2][:, c % 2, :], x_sb[c],
                                       a_all[:, c:c + 1], bb[:, c:c + 1],
                                       OP.mult, OP.add)
                st_["n8", b] = n8

            def qkv(b):
                n8 = st_.pop(("n8", b))
                # q/k into fp8 DoubleRow pair tiles [P, 2, HW]: logical
                # contraction row (2j+i)*128+p lives at [p, i, :] of pair j.
                # Evacuation split DVE (o=0,1) / ACT (o=2,3) so the S matmuls
                # unblock in ~half the serial-evac time.
                for (w_t, tagbase) in ((wq_sb, "q"), (wk_sb, "k")):
                    dst = [wpool.tile([P, 2, HW], F8, tag=f"{tagbase}8{j}",
                                      bufs=2, name=f"{tagbase}8_b{b}_{j}")
                           for j in range(NPAIR)]
                    for o in range(NCH):
                        acc = pp.tile([P, HW], F32, tag="acc2", bufs=3,
                                      name=f"{tagbase}acc_b{b}_{o}")
                        for j in range(NPAIR):
                            for h in range(2):
                                nc.tensor.matmul(
                                    acc[:, h * HWH:(h + 1) * HWH],
                                    lhsT=w_t[j][:, :, o * P:(o + 1) * P],
                                    rhs=n8[j][:, :, h * HWH:(h + 1) * HWH],
                                    start=(j == 0), stop=(j == NPAIR - 1),
                                    perf_mode=DR)
                        out8 = dst[o // 2][:, o % 2, :]
                        if tagbase == "q":
                            nc.vector.tensor_scalar(out8, acc,
                                                    bq16_sb[:, o:o + 1],
                                                    None, OP.add)
                        else:
                            nc.vector.tensor_copy(out=out8, in_=acc)
                    st_[tagbase, b] = dst
                # v-projection interleaved with S^T so the exp chain starts
                # early and finishes before AV needs it.
                v_sb = [wpool.tile([P, 2, HWH], F8, tag=f"v8{j}", bufs=2,
                                   name=f"v8_b{b}_{j}") for j in range(NPT // 2)]
                e_sb = [wpool.tile([P, 2, HW], F8, tag=f"e8{j}",
                                   name=f"e8_b{b}_{j}") for j in range(NPT // 2)]
                q8_sb, k8_sb = st_.pop(("q", b)), st_.pop(("k", b))
                for t8 in range(NPT):
                    vacc = pp.tile([P, HWH], F32, tag="acc1", name=f"vacc_b{b}_{t8}")
                    for j in range(NPAIR):
                        nc.tensor.matmul(vacc,
                                         lhsT=n8[j][:, :, t8 * P:(t8 + 1) * P],
                                         rhs=wv_sb[j],
                                         start=(j == 0), stop=(j == NPAIR - 1),
                                         perf_mode=DR)
                    nc.scalar.copy(v_sb[t8 // 2][:, t8 % 2, :], vacc)

                    m = t8
                    sacc = pp.tile([P, HW], F32, tag="acc2", bufs=3,
                                   name=f"sacc_b{b}_{m}")
                    for j in range(NPAIR):
                        for h in range(2):
                            nc.tensor.matmul(
                                sacc[:, h * HWH:(h + 1) * HWH],
                                lhsT=k8_sb[j][:, :, m * P:(m + 1) * P],
                                rhs=q8_sb[j][:, :, h * HWH:(h + 1) * HWH],
                                start=(j == 0), stop=(j == NPAIR - 1),
                                perf_mode=DR)
                    # scores carry 256x; exp scaled by 1/2 (bias -ln2) for
                    # fp8e4 range headroom; cancels against the denominator.
                    nc.scalar.activation(out=e_sb[m // 2][:, m % 2, :],
                                         in_=sacc, func=AF.Exp, bias=lnh_col,
                                         scale=SCALE / 256.0)
                st_["v", b] = v_sb
                st_["e", b] = e_sb

            def av_den(b):
                e_sb, v_sb = st_["e", b], st_.pop(("v", b))
                o_sb = []
                for ct in range(NCH):
                    acc = pp.tile([P, HW], F32, tag="acc2", bufs=3,
                                  name=f"oacc_b{b}_{ct}")
                    for m in range(NPT // 2):
                        for h in range(2):
                            nc.tensor.matmul(
                                acc[:, h * HWH:(h + 1) * HWH],
                                lhsT=v_sb[m][:, :, ct * P:(ct + 1) * P],
                                rhs=e_sb[m][:, :, h * HWH:(h + 1) * HWH],
                                start=(m == 0), stop=(m == NPT // 2 - 1),
                                perf_mode=DR)
                    j, i = divmod(ct, 2)
                    if i == 0:
                        o_sb.append(wpool.tile([P, 2, HW], F8, tag=f"o8{j}",
                                               name=f"o8_b{b}_{j}"))
                    # 1/64 keeps |attn-raw| inside fp8e4 range (v8 carries
                    # 16x); compensated by the 4.0-valued denominator lhsT.
                    # On DVE: it is idle in this window while ACT's o-copies
                    # would queue behind the 8-exp backlog, delaying proj.
                    nc.vector.tensor_scalar(o_sb[j][:, i, :], acc,
                                            1.0 / 64.0, None, OP.mult)
                st_["o", b] = o_sb
                # 5th channel tile: all-4.0 lhsT sums exp over keys, giving
                # the softmax denominator broadcast to 128 partitions.
                dbc = pp.tile([P, HW], F32, tag="acc2", bufs=3, name=f"dbc_b{b}")
                for m in range(NPT // 2):
                    for h in range(2):
                        nc.tensor.matmul(
                            dbc[:, h * HWH:(h + 1) * HWH],
                            lhsT=ones8m[:, :, :],
                            rhs=e_sb[m][:, :, h * HWH:(h + 1) * HWH],
                            start=(m == 0), stop=(m == NPT // 2 - 1),
                            perf_mode=DR)
                st_.pop(("e", b))
                r_sb = wpool.tile([P, HW], F32, tag="r", name=f"r_b{b}")
                nc.vector.reciprocal_approx_fast(out=r_sb, in_=dbc)
                st_["r", b] = r_sb

            def proj(b):
                o_sb = st_.pop(("o", b))
                x_sb = st_.pop(("x", b))
                oengs = (nc.sync, nc.gpsimd, nc.scalar)
                for o in range(NCH):
                    acc = pp.tile([P, HW], F32, tag="acc2", bufs=3,
                                  name=f"pacc_b{b}_{o}")
                    for j in range(NPAIR):
                        for h in range(2):
                            nc.tensor.matmul(
                                acc[:, h * HWH:(h + 1) * HWH],
                                lhsT=wp_sb[j][:, :, o * P:(o + 1) * P],
                                rhs=o_sb[j][:, :, h * HWH:(h + 1) * HWH],
                                start=(j == 0), stop=(j == NPAIR - 1),
                                perf_mode=DR)
                    if o == 0:
                        r_sb = st_.pop(("r", b))
                    t1 = wpool.tile([P, HW], F32, tag="t1", bufs=2,
                                    name=f"t1_b{b}_{o}")
                    yt = wpool.tile([P, HW], F32, tag=f"y{o}", name=f"y_b{b}_{o}")
                    for h in range(2):
                        sl = slice(h * HWH, (h + 1) * HWH)
                        nc.vector.tensor_mul(t1[:, sl], acc[:, sl], r_sb[:, sl])
                        nc.vector.scalar_tensor_tensor(
                            out=yt[:, sl], in0=t1[:, sl],
                            scalar=bpe_sb[:, o:o + 1], in1=x_sb[o][:, sl],
                            op0=OP.add, op1=OP.add)
                        oeng = oengs[(o * 2 + h) % 3]
                        oeng.dma_start(out=ys_ap[b, o * P:(o + 1) * P, sl],
                                       in_=yt[:, sl])

            # ---- software-pipelined emission, one image ahead ----
            gn_stats(0)
            normalize(0)
            qkv(0)
            for b in range(BPC):
                if b + 1 < BPC:
                    load_x(b + 1)
                av_den(b)
                if b + 1 < BPC:
                    gn_stats(b + 1)
                    normalize(b + 1)
                proj(b)
                if b + 1 < BPC:
                    qkv(b + 1)

    nc.compile()
    return nc


_NC = None


def _get_nc():
    global _NC
    if _NC is None:
        _NC = _build()
    return _NC


def _host_inputs(x, gn_scale, gn_bias, wq, bq, wk, bk, wv, bv, wp, bp):
    x = np.ascontiguousarray(np.asarray(x, np.float32).reshape(B, C, HW))
    f = lambda t: np.ascontiguousarray(np.asarray(t, np.float32))
    gn_scale, gn_bias = f(gn_scale), f(gn_bias)
    bq, bv, bp = f(bq), f(bv), f(bp)
    wq, wk, wv, wp = f(wq), f(wk), f(wv), f(wp)

    bp_eff = bp + wp @ bv  # v-bias passes through softmax-averaging intact
    ch = np.arange(C)
    gmask_full = (ch[:, None] // GS == np.arange(G)[None, :]).astype(np.float32)
    gmask_ = np.ascontiguousarray(gmask_full.reshape(NCH, P, G))
    gmaskT_ = np.zeros((P, C), np.float32)
    gmaskT_[:G, :] = gmask_full.T

    def dr_pack(w):
        wt = np.clip(w.T * WS, -240.0, 240.0).astype(mybir.dt.np(F8))
        wt = wt.reshape(NPAIR, 2, P, C).transpose(0, 2, 1, 3)
        return np.ascontiguousarray(wt)

    common = {
        "wq8d": dr_pack(wq),
        "wk8d": dr_pack(wk),
        "wv8d": dr_pack(wv),
        "wp8d": dr_pack(wp),
        "sbiasd": np.ascontiguousarray(
            np.concatenate([gn_scale.reshape(NCH, P).T,
                            gn_bias.reshape(NCH, P).T], axis=1)),
        "bq16d": np.ascontiguousarray((WS * bq).reshape(NCH, P).T),
        "bped": np.ascontiguousarray(bp_eff.reshape(NCH, P).T),
        "gmask": gmask_,
        "gmaskT": gmaskT_,
        "ones8md": np.full((P, 2, P), 4.0, mybir.dt.np(F8)),
    }
    in_maps = []
    for i in range(NCORES):
        m = dict(common)
        m["xs"] = np.ascontiguousarray(x[i * BPC:(i + 1) * BPC])
        in_maps.append(m)
    return in_maps


def _run(in_maps, trace=False):
    nc = _get_nc()
    return bass_utils.run_bass_kernel_spmd(nc, in_maps, list(range(NCORES)),
                                           trace=trace)


def kernel(**inputs):
    in_maps = _host_inputs(**inputs)
    try:
        res = _run(in_maps, trace=False)
    except Exception:
        # transient device faults (e.g. NRT_EXEC_UNIT_UNRECOVERABLE) clear
        # on re-execution; one retry costs nothing when the first run works
        res = _run(in_maps, trace=False)
    y = np.concatenate([r["ys"] for r in res.results], axis=0)
    return y.reshape(B, C, H, W)


def run_traced(**inputs):
    """Like kernel() but with NTFF tracing; returns (y, exec_time_ns)."""
    in_maps = _host_inputs(**inputs)
    res = _run(in_maps, trace=True)
    y = np.concatenate([r["ys"] for r in res.results], axis=0)
    return y.reshape(B, C, H, W), res.exec_time_ns


# revision 33
# speedup vs baseline: 1.0576x; 1.0576x over previous
"""Trainium2 Bass kernel for GroupNorm + single-head spatial self-attention
(diffusion-style attention block), data-parallel on 8 NeuronCores.

Computation (per image):
    n  = GroupNorm(x; 32 groups) * gn_scale + gn_bias          [C, N]
    q  = wq @ n + bq ; k = wk @ n + bk ; v = wv @ n + bv
    A  = softmax(q^T k / sqrt(C), axis over keys)
    out = x + wp @ (A @ v)^T + bp
Shapes: B=32, C=512, H=W=32 (N = H*W = 1024 positions); 4 images/core.

Design highlights (v2):
  - Measured on this part the PE issues ~259 ns per 512-col matmul at any
    dtype (P0 downclock to ~2.0 GHz; cost = columns x accumulation
    passes).  So EVERYTHING runs fp8e4m3 DoubleRow, which halves the
    number of accumulation passes per 512-deep contraction: per image
    48 q/k/v MMs + 32 scores + 32 AV + 8 denominator + 16 proj
    (vs 192 in the f32r/fp8-mixed v1).
  - All layouts avoid transposes: S^T = k^T q in [keys, queries]; v is
    position-major so AV lands channel-major for the projection.
  - Softmax normalization deferred past AV and the projection:
    y = x + (wp @ AV_raw) * r + bp'.  The denominator is a 5th channel
    tile of AV (all-4.0 DoubleRow lhsT sums exp over keys, broadcast to
    128 partitions); one reciprocal_approx_fast gives r.
  - Scaling chain: n8 = GroupNorm(x) (unit scale), w*8 = 16*w,
    q8/k8/v8 = 16*(q,k,v), scores = 256*S so exp runs at
    scale=C^-0.5/256 with a -ln2 bias; attn-out stored x(1/64); the
    4.0-valued ones lhsT compensates everything through the shared
    denominator.  bk cancels in softmax; bv folds into bp' on host;
    bq is applied x16 at q evacuation.
  - GroupNorm scale/offset vectorized: the 4 group->channel broadcast
    matmuls land in one [128, 8] PSUM tile, and a/b come from 3 DVE ops
    on [128, 4] columns.
  - ~8.5 us of dummy bf16 warm-up matmuls at t=0 hold the PE HAM clock
    gate open through the initial DMA wait (otherwise the first ~25 us
    of real matmuls run at half clock).
  - Emission software-pipelined one image ahead; GroupNorm stats run on
    DVE/ACT/GPSIMD under the previous image's attention matmuls.
"""

import numpy as np

import concourse.bacc as bacc
import concourse.tile as tile
from concourse import mybir
from concourse import bass_utils

F32 = mybir.dt.float32
F8 = mybir.dt.float8e4
BF16 = mybir.dt.bfloat16
DR = mybir.MatmulPerfMode.DoubleRow
LN2 = 0.6931471805599453
AX = mybir.AxisListType.X
OP = mybir.AluOpType
AF = mybir.ActivationFunctionType

B, C, H, W = 32, 512, 32, 32
HW = H * W                      # 1024 spatial positions
HWH = HW // 2                   # 512 = max fp32 matmul free dim
NCORES = 8
BPC = B // NCORES               # images per core
G = 32                          # groups
GS = C // G                     # channels per group
EPS = 1e-5
P = 128
NCH = C // P                    # 4 channel chunks of 128
NPT = HW // P                   # 8 position tiles of 128
NPAIR = NCH // 2                # 2 fp8 DoubleRow channel pairs
SCALE = float(C) ** -0.5
WS = 16.0                       # fp8 weight scale
NWARM = 34                      # dummy warm-up matmuls (N=512 each)


def _build():
    nc = bacc.Bacc("TRN2", target_bir_lowering=False, debug=False)

    xs = nc.dram_tensor("xs", [BPC, C, HW], F32, kind="ExternalInput")
    wq8d = nc.dram_tensor("wq8d", [NPAIR, P, 2, C], F8, kind="ExternalInput")
    wk8d = nc.dram_tensor("wk8d", [NPAIR, P, 2, C], F8, kind="ExternalInput")
    wv8d = nc.dram_tensor("wv8d", [NPAIR, P, 2, C], F8, kind="ExternalInput")
    wp8d = nc.dram_tensor("wp8d", [NPAIR, P, 2, C], F8, kind="ExternalInput")
    # sbias columns: 0-3 gn_scale chunks, 4-7 gn_bias chunks
    sbiasd = nc.dram_tensor("sbiasd", [P, 2 * NCH], F32, kind="ExternalInput")
    bq16d = nc.dram_tensor("bq16d", [P, NCH], F32, kind="ExternalInput")
    bped = nc.dram_tensor("bped", [P, NCH], F32, kind="ExternalInput")
    gmask = nc.dram_tensor("gmask", [NCH, P, G], F32, kind="ExternalInput")
    gmaskT = nc.dram_tensor("gmaskT", [P, C], F32, kind="ExternalInput")
    ones8md = nc.dram_tensor("ones8md", [P, 2, P], F8, kind="ExternalInput")
    ys = nc.dram_tensor("ys", [BPC, C, HW], F32, kind="ExternalOutput")

    xs_ap, ys_ap = xs.ap(), ys.ap()

    with tile.TileContext(nc) as tc:
        with (
            tc.tile_pool(name="consts", bufs=1) as cp,
            tc.tile_pool(name="work", bufs=1) as wpool,
            tc.tile_pool(name="psum", bufs=2, space="PSUM") as pp,
        ):
            st_ = {}   # mutable per-image state keyed (name, b)

            # ---- image-0 x load first so GN starts before weights land ----
            def load_x(b):
                tiles = []
                engs = (nc.sync, nc.gpsimd, nc.scalar)
                for c in range(NCH):
                    xt = wpool.tile([P, HW], F32, tag=f"x{c}", bufs=2,
                                    name=f"x_b{b}_{c}")
                    engs[c % 3].dma_start(
                        out=xt, in_=xs_ap[b, c * P:(c + 1) * P, :])
                    tiles.append(xt)
                st_["x", b] = tiles

            load_x(0)

            # ---- warm-up source (no DMA dependency) + HAM hold-open ----
            wsrc = cp.tile([P, HWH], F32, tag="wsrc", name="wsrc")
            nc.vector.memset(wsrc, 0.125)
            wlhs = cp.tile([P, P], BF16, tag="wlhs", name="wlhs")
            nc.vector.tensor_copy(out=wlhs, in_=wsrc[:, :P])
            wrhs = cp.tile([P, HWH], BF16, tag="wrhs", name="wrhs")
            nc.vector.tensor_copy(out=wrhs, in_=wsrc)
            warm = pp.tile([P, HWH], F32, tag="acc1", name="warm")
            for _ in range(NWARM):
                nc.tensor.matmul(warm, lhsT=wlhs, rhs=wrhs,
                                 start=True, stop=True)

            # ---- constants ----
            def const_w8(dram, tagbase):
                tiles = []
                for j in range(NPAIR):
                    t = cp.tile([P, 2, C], F8, tag=f"{tagbase}{j}",
                                name=f"{tagbase}{j}")
                    eng = nc.sync if j % 2 == 0 else nc.gpsimd
                    eng.dma_start(out=t, in_=dram.ap()[j])
                    tiles.append(t)
                return tiles

            gm_sb = []
            for c in range(NCH):
                t = cp.tile([P, G], F32, tag=f"gm{c}", name=f"gm{c}")
                nc.sync.dma_start(out=t, in_=gmask.ap()[c])
                gm_sb.append(t)
            gmT_sb = cp.tile([P, C], F32, tag="gmT", name="gmT")
            nc.sync.dma_start(out=gmT_sb, in_=gmaskT.ap())
            sbias_sb = cp.tile([P, 2 * NCH], F32, tag="sbias", name="sbias")
            nc.gpsimd.dma_start(out=sbias_sb, in_=sbiasd.ap())
            bq16_sb = cp.tile([P, NCH], F32, tag="bq16", name="bq16")
            nc.gpsimd.dma_start(out=bq16_sb, in_=bq16d.ap())
            bpe_sb = cp.tile([P, NCH], F32, tag="bpe", name="bpe")
            nc.gpsimd.dma_start(out=bpe_sb, in_=bped.ap())
            eps_sb = cp.tile([P, 1], F32, tag="eps", name="eps")
            nc.vector.memset(eps_sb, EPS)
            zero_col = cp.tile([P, 1], F32, tag="zero", name="zero")
            nc.vector.memset(zero_col, 0.0)
            lnh_col = cp.tile([P, 1], F32, tag="lnh", name="lnh")
            nc.vector.memset(lnh_col, -LN2)

            wq_sb = const_w8(wq8d, "wq")
            wk_sb = const_w8(wk8d, "wk")
            wv_sb = const_w8(wv8d, "wv")
            wp_sb = const_w8(wp8d, "wp")
            ones8m = cp.tile([P, 2, P], F8, tag="ones8m", name="ones8m")
            nc.sync.dma_start(out=ones8m, in_=ones8md.ap())

            # ---- per-image phases ----
            def gn_stats(b):
                x_sb = st_["x", b]
                stt = []
                for c in range(NCH):
                    s = wpool.tile([P, 2], F32, tag=f"st{c}", name=f"st_b{b}_{c}")
                    nc.vector.reduce_sum(out=s[:, 0:1], in_=x_sb[c], axis=AX)
                    scr = wpool.tile([P, HW], F32, tag="sqscr", bufs=2,
                                     name=f"sqscr_b{b}_{c}")
                    nc.scalar.activation(out=scr, in_=x_sb[c], func=AF.Square,
                                         bias=zero_col, accum_out=s[:, 1:2])
                    stt.append(s)

                gp = pp.tile([G, 2], F32, tag="acc1", name=f"gp_b{b}")
                for c in range(NCH):
                    nc.tensor.matmul(gp, lhsT=gm_sb[c], rhs=stt[c],
                                     start=(c == 0), stop=(c == NCH - 1))

                # gmr: col0 = group mean, col1 = group rstd (rows >= G zero)
                gmr = wpool.tile([P, 2], F32, tag="gmr", name=f"gmr_b{b}")
                nc.vector.memset(gmr, 0.0)
                nc.vector.tensor_scalar(gmr[:G, 0:1], gp[:G, 0:1],
                                        1.0 / (GS * HW), None, OP.mult)
                m2 = wpool.tile([P, 1], F32, tag="m2", name=f"m2_b{b}")
                nc.vector.tensor_mul(m2[:G], gmr[:G, 0:1], gmr[:G, 0:1])
                var = wpool.tile([P, 1], F32, tag="var", name=f"var_b{b}")
                nc.vector.scalar_tensor_tensor(
                    out=var[:G], in0=gp[:G, 1:2], scalar=1.0 / (GS * HW),
                    in1=m2[:G], op0=OP.mult, op1=OP.subtract)
                sd = wpool.tile([P, 1], F32, tag="sd", name=f"sd_b{b}")
                nc.scalar.activation(out=sd[:G], in_=var[:G],
                                     func=AF.Sqrt, bias=eps_sb[:G])
                nc.vector.reciprocal(out=gmr[:G, 1:2], in_=sd[:G])
                st_["gmr", b] = gmr

            def normalize(b):
                x_sb, gmr = st_["x", b], st_.pop(("gmr", b))
                # one [128, 8] PSUM tile: cols (2c, 2c+1) = per-channel
                # (mean, rstd) for chunk c
                bcm = pp.tile([P, 2 * NCH], F32, tag="acc1",
                              name=f"bcm_b{b}")
                for c in range(NCH):
                    nc.tensor.matmul(bcm[:, 2 * c:2 * c + 2],
                                     lhsT=gmT_sb[:, c * P:(c + 1) * P],
                                     rhs=gmr, start=True, stop=True)
                a_all = wpool.tile([P, NCH], F32, tag="a_all",
                                   name=f"a_b{b}")
                nc.vector.tensor_mul(a_all, bcm[:, 1:2 * NCH:2],
                                     sbias_sb[:, 0:NCH])
                gt = wpool.tile([P, NCH], F32, tag="gt", name=f"gt_b{b}")
                nc.vector.tensor_mul(gt, bcm[:, 0:2 * NCH:2], a_all)
                bb = wpool.tile([P, NCH], F32, tag="bb", name=f"bb_b{b}")
                nc.vector.tensor_sub(bb, sbias_sb[:, NCH:2 * NCH], gt)
                n8 = [wpool.tile([P, 2, HW], F8, tag=f"n8{j}", bufs=2,
                                 name=f"n8_b{b}_{j}") for j in range(NPAIR)]
                for c in range(NCH):
                    neng = nc.vector if b == 0 else nc.gpsimd
                    neng.tensor_scalar(n8[c // 2][:, c % 2, :], x_sb[c],
                                       a_all[:, c:c + 1], bb[:, c:c + 1],
                                       OP.mult, OP.add)
                st_["n8", b] = n8

            def qkv(b):
                n8 = st_.pop(("n8", b))
                # q/k into fp8 DoubleRow pair tiles [P, 2, HW]: logical
                # contraction row (2j+i)*128+p lives at [p, i, :] of pair j.
                # Evacuation split DVE (o=0,1) / ACT (o=2,3) so the S matmuls
                # unblock in ~half the serial-evac time.
                for (w_t, tagbase) in ((wq_sb, "q"), (wk_sb, "k")):
                    dst = [wpool.tile([P, 2, HW], F8, tag=f"{tagbase}8{j}",
                                      bufs=2, name=f"{tagbase}8_b{b}_{j}")
                           for j in range(NPAIR)]
                    for o in range(NCH):
                        acc = pp.tile([P, HW], F32, tag="acc2", bufs=3,
                                      name=f"{tagbase}acc_b{b}_{o}")
                        for j in range(NPAIR):
                            for h in range(2):
                                nc.tensor.matmul(
                                    acc[:, h * HWH:(h + 1) * HWH],
                                    lhsT=w_t[j][:, :, o * P:(o + 1) * P],
                                    rhs=n8[j][:, :, h * HWH:(h + 1) * HWH],
                                    start=(j == 0), stop=(j == NPAIR - 1),
                                    perf_mode=DR)
                        out8 = dst[o // 2][:, o % 2, :]
                        if tagbase == "q":
                            nc.vector.tensor_scalar(out8, acc,
                                                    bq16_sb[:, o:o + 1],
                                                    None, OP.add)
                        else:
                            nc.vector.tensor_copy(out=out8, in_=acc)
                    st_[tagbase, b] = dst
                # v-projection interleaved with S^T so the exp chain starts
                # early and finishes before AV needs it.
                v_sb = [wpool.tile([P, 2, HWH], F8, tag=f"v8{j}", bufs=2,
                                   name=f"v8_b{b}_{j}") for j in range(NPT // 2)]
                e_sb = [wpool.tile([P, 2, HW], F8, tag=f"e8{j}",
                                   name=f"e8_b{b}_{j}") for j in range(NPT // 2)]
                q8_sb, k8_sb = st_.pop(("q", b)), st_.pop(("k", b))
                for t8 in range(NPT):
                    vacc = pp.tile([P, HWH], F32, tag="acc1", name=f"vacc_b{b}_{t8}")
                    for j in range(NPAIR):
                        nc.tensor.matmul(vacc,
                                         lhsT=n8[j][:, :, t8 * P:(t8 + 1) * P],
                                         rhs=wv_sb[j],
                                         start=(j == 0), stop=(j == NPAIR - 1),
                                         perf_mode=DR)
                    nc.scalar.copy(v_sb[t8 // 2][:, t8 % 2, :], vacc)

                    m = t8
                    sacc = pp.tile([P, HW], F32, tag="acc2", bufs=3,
                                   name=f"sacc_b{b}_{m}")
                    for j in range(NPAIR):
                        for h in range(2):
                            nc.tensor.matmul(
                                sacc[:, h * HWH:(h + 1) * HWH],
                                lhsT=k8_sb[j][:, :, m * P:(m + 1) * P],
                                rhs=q8_sb[j][:, :, h * HWH:(h + 1) * HWH],
                                start=(j == 0), stop=(j == NPAIR - 1),
                                perf_mode=DR)
                    # scores carry 256x; exp scaled by 1/2 (bias -ln2) for
                    # fp8e4 range headroom; cancels against the denominator.
                    nc.scalar.activation(out=e_sb[m // 2][:, m % 2, :],
                                         in_=sacc, func=AF.Exp, bias=lnh_col,
                                         scale=SCALE / 256.0)
                st_["v", b] = v_sb
                st_["e", b] = e_sb

            def av_den(b):
                e_sb, v_sb = st_["e", b], st_.pop(("v", b))
                o_sb = []
                for ct in range(NCH):
                    acc = pp.tile([P, HW], F32, tag="acc2", bufs=3,
                                  name=f"oacc_b{b}_{ct}")
                    for m in range(NPT // 2):
                        for h in range(2):
                            nc.tensor.matmul(
                                acc[:, h * HWH:(h + 1) * HWH],
                                lhsT=v_sb[m][:, :, ct * P:(ct + 1) * P],
                                rhs=e_sb[m][:, :, h * HWH:(h + 1) * HWH],
                                start=(m == 0), stop=(m == NPT // 2 - 1),
                                perf_mode=DR)
                    j, i = divmod(ct, 2)
                    if i == 0:
                        o_sb.append(wpool.tile([P, 2, HW], F8, tag=f"o8{j}",
                                               name=f"o8_b{b}_{j}"))
                    # 1/64 keeps |attn-raw| inside fp8e4 range (v8 carries
                    # 16x); compensated by the 4.0-valued denominator lhsT.
                    nc.scalar.activation(out=o_sb[j][:, i, :], in_=acc,
                                         func=AF.Copy, scale=1.0 / 64.0)
                st_["o", b] = o_sb
                # 5th channel tile: all-4.0 lhsT sums exp over keys, giving
                # the softmax denominator broadcast to 128 partitions.
                dbc = pp.tile([P, HW], F32, tag="acc2", bufs=3, name=f"dbc_b{b}")
                for m in range(NPT // 2):
                    for h in range(2):
                        nc.tensor.matmul(
                            dbc[:, h * HWH:(h + 1) * HWH],
                            lhsT=ones8m[:, :, :],
                            rhs=e_sb[m][:, :, h * HWH:(h + 1) * HWH],
                            start=(m == 0), stop=(m == NPT // 2 - 1),
                            perf_mode=DR)
                st_.pop(("e", b))
                r_sb = wpool.tile([P, HW], F32, tag="r", name=f"r_b{b}")
                nc.vector.reciprocal_approx_fast(out=r_sb, in_=dbc)
                st_["r", b] = r_sb

            def proj(b):
                o_sb = st_.pop(("o", b))
                x_sb = st_.pop(("x", b))
                oengs = (nc.sync, nc.gpsimd, nc.scalar)
                for o in range(NCH):
                    acc = pp.tile([P, HW], F32, tag="acc2", bufs=3,
                                  name=f"pacc_b{b}_{o}")
                    for j in range(NPAIR):
                        for h in range(2):
                            nc.tensor.matmul(
                                acc[:, h * HWH:(h + 1) * HWH],
                                lhsT=wp_sb[j][:, :, o * P:(o + 1) * P],
                                rhs=o_sb[j][:, :, h * HWH:(h + 1) * HWH],
                                start=(j == 0), stop=(j == NPAIR - 1),
                                perf_mode=DR)
                    if o == 0:
                        r_sb = st_.pop(("r", b))
                    t1 = wpool.tile([P, HW], F32, tag="t1", bufs=2,
                                    name=f"t1_b{b}_{o}")
                    yt = wpool.tile([P, HW], F32, tag=f"y{o}", name=f"y_b{b}_{o}")
                    for h in range(2):
                        sl = slice(h * HWH, (h + 1) * HWH)
                        nc.vector.tensor_mul(t1[:, sl], acc[:, sl], r_sb[:, sl])
                        nc.vector.scalar_tensor_tensor(
                            out=yt[:, sl], in0=t1[:, sl],
                            scalar=bpe_sb[:, o:o + 1], in1=x_sb[o][:, sl],
                            op0=OP.add, op1=OP.add)
                        oeng = oengs[(o * 2 + h) % 3]
                        oeng.dma_start(out=ys_ap[b, o * P:(o + 1) * P, sl],
                                       in_=yt[:, sl])

            # ---- software-pipelined emission, one image ahead ----
            gn_stats(0)
            normalize(0)
            qkv(0)
            for b in range(BPC):
                if b + 1 < BPC:
                    load_x(b + 1)
                av_den(b)
                if b + 1 < BPC:
                    gn_stats(b + 1)
                    normalize(b + 1)
                proj(b)
                if b + 1 < BPC:
                    qkv(b + 1)

    nc.compile()
    return nc


_NC = None


def _get_nc():
    global _NC
    if _NC is None:
        _NC = _build()
    return _NC


def _host_inputs(x, gn_scale, gn_bias, wq, bq, wk, bk, wv, bv, wp, bp):
    x = np.ascontiguousarray(np.asarray(x, np.float32).reshape(B, C, HW))
    f = lambda t: np.ascontiguousarray(np.asarray(t, np.float32))
    gn_scale, gn_bias = f(gn_scale), f(gn_bias)
    bq, bv, bp = f(bq), f(bv), f(bp)
    wq, wk, wv, wp = f(wq), f(wk), f(wv), f(wp)

    bp_eff = bp + wp @ bv  # v-bias passes through softmax-averaging intact
    ch = np.arange(C)
    gmask_full = (ch[:, None] // GS == np.arange(G)[None, :]).astype(np.float32)
    gmask_ = np.ascontiguousarray(gmask_full.reshape(NCH, P, G))
    gmaskT_ = np.zeros((P, C), np.float32)
    gmaskT_[:G, :] = gmask_full.T

    def dr_pack(w):
        wt = np.clip(w.T * WS, -240.0, 240.0).astype(mybir.dt.np(F8))
        wt = wt.reshape(NPAIR, 2, P, C).transpose(0, 2, 1, 3)
        return np.ascontiguousarray(wt)

    common = {
        "wq8d": dr_pack(wq),
        "wk8d": dr_pack(wk),
        "wv8d": dr_pack(wv),
        "wp8d": dr_pack(wp),
        "sbiasd": np.ascontiguousarray(
            np.concatenate([gn_scale.reshape(NCH, P).T,
                            gn_bias.reshape(NCH, P).T], axis=1)),
        "bq16d": np.ascontiguousarray((WS * bq).reshape(NCH, P).T),
        "bped": np.ascontiguousarray(bp_eff.reshape(NCH, P).T),
        "gmask": gmask_,
        "gmaskT": gmaskT_,
        "ones8md": np.full((P, 2, P), 4.0, mybir.dt.np(F8)),
    }
    in_maps = []
    for i in range(NCORES):
        m = dict(common)
        m["xs"] = np.ascontiguousarray(x[i * BPC:(i + 1) * BPC])
        in_maps.append(m)
    return in_maps


def _run(in_maps, trace=False):
    nc = _get_nc()
    return bass_utils.run_bass_kernel_spmd(nc, in_maps, list(range(NCORES)),
                                           trace=trace)


def kernel(**inputs):
    in_maps = _host_inputs(**inputs)
    try:
        res = _run(in_maps, trace=False)
    except Exception:
        # transient device faults (e.g. NRT_EXEC_UNIT_UNRECOVERABLE) clear
        # on re-execution; one retry costs nothing when the first run works
        res = _run(in_maps, trace=False)
    y = np.concatenate([r["ys"] for r in res.results], axis=0)
    return y.reshape(B, C, H, W)


def run_traced(**inputs):
    """Like kernel() but with NTFF tracing; returns (y, exec_time_ns)."""
    in_maps = _host_inputs(**inputs)
    res = _run(in_maps, trace=True)
    y = np.concatenate([r["ys"] for r in res.results], axis=0)
    return y.reshape(B, C, H, W), res.exec_time_ns
